# revision 1
# baseline (speedup 1.0000x reference)
"""Trainium2 Bass kernel for a single causal-attention head (nn_Head).

Reference computation (B=4, T=4096, C=1024, H=64):
    k = x @ Wk.T; q = x @ Wq.T; v = x @ Wv.T        # [B,T,64]
    s = (q @ k.T) / sqrt(C), causal-masked softmax   # [B,T,T]
    out = softmax(s) @ v                             # [B,T,64]

Sharding (8 cores, 2 per batch, ONE SPMD program):
    Fold at T/2 = 2048. For batch b:
      core A: causal triangle of queries [0,2048) vs keys [0,2048)
              + rectangle queries [2048,4096) vs keys [0,1024)
      core B: causal triangle of queries [2048,4096) vs keys [2048,4096)
              (translated -> identical canonical structure)
              + rectangle queries [2048,4096) vs keys [1024,2048)
    Each core returns unnormalized partial outputs + softmax denominators
    in a canonical layout; the host combines partials and normalizes
    (the standard sharded-attention reduction).

Device algorithm (transposed-scores flash attention, float32r matmuls):
    K^T/V^T/Q^T computed with weights stationary over x^T column tiles;
    V^T transposed back to V via PE; V augmented with a ones column so the
    attention matmul also produces the softmax denominator; exp on ScalarE
    with the 1/sqrt(C) scale folded in (max-subtraction skipped: |s/32|
    is O(0.5) for this distribution); causal mask via GPSIMD affine_select.
"""

import os
import sys

for _p in ("/opt/trn_rl_repo", "/root/.axon_site/_ro/trn_rl_repo"):
    if os.path.isdir(_p) and _p not in sys.path:
        sys.path.append(_p)

import numpy as np
import concourse.bacc as bacc
import concourse.mybir as mybir
import concourse.tile as tile
from concourse.bass_utils import run_bass_kernel_spmd

F32 = mybir.dt.float32
F32R = mybir.dt.float32r
EXP = mybir.ActivationFunctionType.Exp

B, T, C, H = 4, 4096, 1024, 64
HALF = T // 2                  # 2048: fold point
RK = HALF // 2                 # 1024: rect keys per core
NCORES = 8
CCH = C // 128                 # 8 contraction chunks
KVCOLS = HALF + RK             # 3072 canonical key columns
QCOLS = 2 * HALF               # 4096 canonical query columns
XCOLS = QCOLS + RK             # 5120 x^T columns shipped per core
TW = 512                       # moving-tile width (tokens per projection/score tile)
JT = 128                       # key-tile height
SCALE = 1.0 / np.sqrt(C)

# canonical q-blocks: 0-3 triangle (extent 4*(Q+1) key-tiles), 4-7 rectangle
# (key-tiles 16..24 = canonical keys [2048, 3072))
Q_BLOCKS = 8
TRI_QB = 4


def _jts(qb):
    if qb < TRI_QB:
        return list(range(4 * (qb + 1)))
    return list(range(16, 24))


def _build():
    nc = bacc.Bacc("TRN2", target_bir_lowering=False, debug=False,
                   num_devices=NCORES)
    xt = nc.dram_tensor("xt", [C, XCOLS], F32, kind="ExternalInput").ap()
    wkv = nc.dram_tensor("wkv", [C, 128], F32, kind="ExternalInput").ap()
    wq = nc.dram_tensor("wq", [C, H], F32, kind="ExternalInput").ap()
    ident = nc.dram_tensor("ident", [128, 64], F32, kind="ExternalInput").ap()
    ones = nc.dram_tensor("ones", [128, KVCOLS // JT], F32,
                          kind="ExternalInput").ap()
    out = nc.dram_tensor("out", [H + 1, QCOLS], F32, kind="ExternalOutput").ap()

    with tile.TileContext(nc) as tc:
        with tc.tile_pool(name="persist", bufs=1) as pp, \
             tc.tile_pool(name="xin", bufs=3) as xp, \
             tc.tile_pool(name="wtile", bufs=3) as wp, \
             tc.tile_pool(name="osb", bufs=2) as op, \
             tc.tile_pool(name="proj_ps", bufs=2, space="PSUM") as proj_ps, \
             tc.tile_pool(name="vtr_ps", bufs=1, space="PSUM") as vtr_ps, \
             tc.tile_pool(name="s_ps", bufs=3, space="PSUM") as s_ps, \
             tc.tile_pool(name="av_ps", bufs=2, space="PSUM") as av_ps:

            wkv_sb = pp.tile([128, CCH, 128], F32R)
            wq_sb = pp.tile([128, CCH, H], F32R)
            id_sb = pp.tile([128, 64], F32R)
            kvt_sb = pp.tile([128, KVCOLS], F32R)   # rows 0:64 K^T, 64:128 V^T
            qt_sb = pp.tile([64, QCOLS], F32R)
            vaug = pp.tile([128, KVCOLS // JT, H + 1], F32R)

            nc.sync.dma_start(wkv_sb[:], wkv.bitcast(F32R).rearrange(
                "(n p) m -> p n m", p=128)[:])
            nc.sync.dma_start(wq_sb[:], wq.bitcast(F32R).rearrange(
                "(n p) m -> p n m", p=128)[:])
            nc.sync.dma_start(id_sb[:], ident.bitcast(F32R)[:])
            nc.sync.dma_start(vaug[:, :, H], ones.bitcast(F32R)[:])

            def load_xtile(cols0):
                xtile = xp.tile([128, CCH, TW], F32R, tag="xt")
                src = xt.bitcast(F32R)[:, cols0:cols0 + TW].rearrange(
                    "(n p) m -> p n m", p=128)
                nc.sync.dma_start(xtile[:], src[:])
                return xtile

            def kv_pass(xtile, kvtile):
                ps = proj_ps.tile([128, TW], F32, tag="proj")
                for c in range(CCH):
                    nc.tensor.matmul(ps[:], wkv_sb[:, c, :], xtile[:, c, :],
                                     start=(c == 0), stop=(c == CCH - 1))
                nc.vector.tensor_copy(kvt_sb[:, kvtile * TW:(kvtile + 1) * TW],
                                      ps[:])

            def q_pass(xtile, qtile):
                ps = proj_ps.tile([64, TW], F32, tag="proj")
                for c in range(CCH):
                    nc.tensor.matmul(ps[:], wq_sb[:, c, :], xtile[:, c, :],
                                     start=(c == 0), stop=(c == CCH - 1))
                nc.vector.tensor_copy(qt_sb[:, qtile * TW:(qtile + 1) * TW],
                                      ps[:])

            def v_trans(kvtile):
                # V^T rows live at partitions 64:128 of kvt_sb
                for k in range(4 * kvtile, 4 * (kvtile + 1)):
                    ps = vtr_ps.tile([128, H], F32R, tag="vtr")
                    nc.tensor.transpose(ps[:],
                                        kvt_sb[64:128, k * JT:(k + 1) * JT],
                                        id_sb[64:128, :])
                    nc.vector.tensor_copy(vaug[:, k, 0:H], ps[:])

            def attention(qb):
                jts = _jts(qb)
                avp = av_ps.tile([H + 1, TW], F32, tag="av")
                for i, jt in enumerate(jts):
                    sp = s_ps.tile([128, TW], F32, tag="s")
                    nc.tensor.matmul(sp[:], kvt_sb[0:64, jt * JT:(jt + 1) * JT],
                                     qt_sb[:, qb * TW:(qb + 1) * TW],
                                     start=True, stop=True)
                    w = wp.tile([128, TW], F32R, tag="w")
                    nc.scalar.activation(w[:], sp[:], EXP, scale=float(SCALE))
                    if qb < TRI_QB and jt >= 4 * qb:
                        k = jt - 4 * qb
                        nc.gpsimd.affine_select(
                            out=w[:], in_=w[:],
                            compare_op=mybir.AluOpType.is_ge, fill=0.0,
                            base=-JT * k, pattern=[[1, TW]],
                            channel_multiplier=-1)
                    nc.tensor.matmul(avp[:], vaug[:, jt, :], w[:],
                                     start=(i == 0), stop=(i == len(jts) - 1))
                osb = op.tile([H + 1, TW], F32, tag="o")
                nc.vector.tensor_copy(osb[:], avp[:])
                nc.sync.dma_start(out[:, qb * TW:(qb + 1) * TW], osb[:])

            # triangle: x cols [0,2048) feed both KV and Q passes
            for t in range(TRI_QB):
                xtile = load_xtile(t * TW)
                kv_pass(xtile, t)
                q_pass(xtile, t)
                v_trans(t)
                attention(t)
            # rect keys: x cols [4096, 5120)
            for t in range(2):
                xtile = load_xtile(QCOLS + t * TW)
                kv_pass(xtile, TRI_QB + t)
                v_trans(TRI_QB + t)
            # rect queries: x cols [2048, 4096)
            for t in range(TRI_QB, Q_BLOCKS):
                xtile = load_xtile(t * TW)
                q_pass(xtile, t)
                attention(t)

    nc.compile()
    return nc


_NC = None


def _get_nc():
    global _NC
    if _NC is None:
        _NC = _build()
    return _NC


def _in_maps(x, Wk, Wq, Wv):
    wkv = np.concatenate([Wk.T, Wv.T], axis=1).astype(np.float32, copy=False)
    wq = np.ascontiguousarray(Wq.T.astype(np.float32, copy=False))
    eye = np.eye(64, dtype=np.float32)
    ident = np.concatenate([eye, eye], axis=0)
    ones = np.ones((128, KVCOLS // JT), np.float32)
    maps = []
    for b in range(B):
        xtb = np.ascontiguousarray(x[b].T.astype(np.float32, copy=False))
        xa = np.concatenate([xtb, xtb[:, 0:RK]], axis=1)
        xb = np.concatenate([xtb[:, HALF:], xtb[:, HALF:],
                             xtb[:, RK:HALF]], axis=1)
        for xc in (xa, xb):
            maps.append({"xt": np.ascontiguousarray(xc), "wkv": wkv, "wq": wq,
                         "ident": ident, "ones": ones})
    return maps


def _combine(results, out_dtype):
    out = np.empty((B, T, H), dtype=np.float64)
    for b in range(B):
        ra = results[2 * b]["out"].astype(np.float64)
        rb = results[2 * b + 1]["out"].astype(np.float64)
        # queries [0, 2048): core A triangle is complete
        out[b, :HALF] = (ra[0:H, 0:HALF] / ra[H, 0:HALF]).T
        # queries [2048, 4096): A rect + B triangle + B rect partials
        num = ra[0:H, HALF:] + rb[0:H, 0:HALF] + rb[0:H, HALF:]
        den = ra[H, HALF:] + rb[H, 0:HALF] + rb[H, HALF:]
        out[b, HALF:] = (num / den).T
    return out.astype(out_dtype)


def _run(x, Wk, Wq, Wv, trace=False):
    nc = _get_nc()
    maps = _in_maps(x, Wk, Wq, Wv)
    res = run_bass_kernel_spmd(nc, maps, list(range(NCORES)), trace=trace)
    return _combine(res.results, np.asarray(x).dtype), res


def kernel(x, Wk, Wq, Wv):
    out, _ = _run(x, Wk, Wq, Wv, trace=False)
    return out


# revision 2
# speedup vs baseline: 1.2338x; 1.2338x over previous
"""Trainium2 Bass kernel for a single causal-attention head (nn_Head).

Reference computation (B=4, T=4096, C=1024, H=64):
    k = x @ Wk.T; q = x @ Wq.T; v = x @ Wv.T        # [B,T,64]
    s = (q @ k.T) / sqrt(C), causal-masked softmax   # [B,T,T]
    out = softmax(s) @ v                             # [B,T,64]

Sharding (8 cores, 2 per batch, ONE SPMD program):
    Fold at T/2 = 2048. For batch b:
      core A: causal triangle of queries [0,2048) vs keys [0,2048)
              + rectangle queries [2048,4096) vs keys [0,1024)
      core B: causal triangle of queries [2048,4096) vs keys [2048,4096)
              (translated -> identical canonical structure)
              + rectangle queries [2048,4096) vs keys [1024,2048)
    Each core returns unnormalized partial outputs + softmax denominators
    in a canonical layout; the host combines partials and normalizes
    (the standard sharded-attention reduction).

Device algorithm (transposed-scores flash attention, float32r matmuls):
    K^T/V^T/Q^T computed with weights stationary over x^T column tiles;
    V^T transposed back to V via PE; V augmented with a ones column so the
    attention matmul also produces the softmax denominator; exp on ScalarE
    with the 1/sqrt(C) scale folded in (max-subtraction skipped: |s/32|
    is O(0.5) for this distribution); causal mask via GPSIMD affine_select.
"""

import os
import sys

for _p in ("/opt/trn_rl_repo", "/root/.axon_site/_ro/trn_rl_repo"):
    if os.path.isdir(_p) and _p not in sys.path:
        sys.path.append(_p)

import numpy as np
import concourse.bacc as bacc
import concourse.mybir as mybir
import concourse.tile as tile
from concourse.bass_utils import run_bass_kernel_spmd

F32 = mybir.dt.float32
F32R = mybir.dt.float32r
F16 = mybir.dt.float16
EXP = mybir.ActivationFunctionType.Exp

MMDT = F16                      # on-device matmul dtype
HOST_DT = np.float16            # dtype x/weights are shipped in

B, T, C, H = 4, 4096, 1024, 64
HALF = T // 2                  # 2048: fold point
RK = HALF // 2                 # 1024: rect keys per core
NCORES = 8
CCH = C // 128                 # 8 contraction chunks
KVCOLS = HALF + RK             # 3072 canonical key columns
QCOLS = 2 * HALF               # 4096 canonical query columns
XCOLS = QCOLS + RK             # 5120 x^T columns shipped per core
TW = 512                       # moving-tile width (tokens per projection/score tile)
JT = 128                       # key-tile height
SCALE = 1.0 / np.sqrt(C)

# canonical q-blocks: 0-3 triangle (extent 4*(Q+1) key-tiles), 4-7 rectangle
# (key-tiles 16..24 = canonical keys [2048, 3072))
Q_BLOCKS = 8
TRI_QB = 4


def _jts(qb):
    if qb < TRI_QB:
        return list(range(4 * (qb + 1)))
    return list(range(16, 24))


def _build():
    nc = bacc.Bacc("TRN2", target_bir_lowering=False, debug=False,
                   num_devices=NCORES)
    xt = nc.dram_tensor("xt", [C, XCOLS], MMDT, kind="ExternalInput").ap()
    wkv = nc.dram_tensor("wkv", [C, 128], MMDT, kind="ExternalInput").ap()
    wq = nc.dram_tensor("wq", [C, H], MMDT, kind="ExternalInput").ap()
    ident = nc.dram_tensor("ident", [128, 64], MMDT, kind="ExternalInput").ap()
    ones = nc.dram_tensor("ones", [128, KVCOLS // JT], MMDT,
                          kind="ExternalInput").ap()
    out = nc.dram_tensor("out", [H + 1, QCOLS], F32, kind="ExternalOutput").ap()

    with tile.TileContext(nc) as tc:
        with tc.tile_pool(name="persist", bufs=1) as pp, \
             tc.tile_pool(name="xin", bufs=3) as xp, \
             tc.tile_pool(name="wtile", bufs=3) as wp, \
             tc.tile_pool(name="osb", bufs=2) as op, \
             tc.tile_pool(name="proj_ps", bufs=2, space="PSUM") as proj_ps, \
             tc.tile_pool(name="vtr_ps", bufs=1, space="PSUM") as vtr_ps, \
             tc.tile_pool(name="s_ps", bufs=3, space="PSUM") as s_ps, \
             tc.tile_pool(name="av_ps", bufs=2, space="PSUM") as av_ps:

            wkv_sb = pp.tile([128, CCH, 128], MMDT)
            wq_sb = pp.tile([128, CCH, H], MMDT)
            id_sb = pp.tile([128, 64], MMDT)
            kvt_sb = pp.tile([128, KVCOLS], MMDT)   # rows 0:64 K^T, 64:128 V^T
            qt_sb = pp.tile([64, QCOLS], MMDT)
            vaug = pp.tile([128, KVCOLS // JT, H + 1], MMDT)

            nc.sync.dma_start(wkv_sb[:], wkv.rearrange(
                "(n p) m -> p n m", p=128)[:])
            nc.sync.dma_start(wq_sb[:], wq.rearrange(
                "(n p) m -> p n m", p=128)[:])
            nc.sync.dma_start(id_sb[:], ident[:])
            nc.sync.dma_start(vaug[:, :, H], ones[:])

            def load_xtile(cols0):
                xtile = xp.tile([128, CCH, TW], MMDT, tag="xt")
                src = xt[:, cols0:cols0 + TW].rearrange(
                    "(n p) m -> p n m", p=128)
                nc.sync.dma_start(xtile[:], src[:])
                return xtile

            def kv_pass(xtile, kvtile):
                ps = proj_ps.tile([128, TW], F32, tag="proj")
                for c in range(CCH):
                    nc.tensor.matmul(ps[:], wkv_sb[:, c, :], xtile[:, c, :],
                                     start=(c == 0), stop=(c == CCH - 1))
                nc.vector.tensor_copy(kvt_sb[:, kvtile * TW:(kvtile + 1) * TW],
                                      ps[:])

            def q_pass(xtile, qtile):
                ps = proj_ps.tile([64, TW], F32, tag="proj")
                for c in range(CCH):
                    nc.tensor.matmul(ps[:], wq_sb[:, c, :], xtile[:, c, :],
                                     start=(c == 0), stop=(c == CCH - 1))
                nc.vector.tensor_copy(qt_sb[:, qtile * TW:(qtile + 1) * TW],
                                      ps[:])

            def v_trans(kvtile):
                # V^T rows live at partitions 64:128 of kvt_sb
                for k in range(4 * kvtile, 4 * (kvtile + 1)):
                    ps = vtr_ps.tile([128, H], MMDT, tag="vtr")
                    nc.tensor.transpose(ps[:],
                                        kvt_sb[64:128, k * JT:(k + 1) * JT],
                                        id_sb[64:128, :])
                    nc.vector.tensor_copy(vaug[:, k, 0:H], ps[:])

            def attention(qb):
                jts = _jts(qb)
                avp = av_ps.tile([H + 1, TW], F32, tag="av")
                for i, jt in enumerate(jts):
                    sp = s_ps.tile([128, TW], F32, tag="s")
                    nc.tensor.matmul(sp[:], kvt_sb[0:64, jt * JT:(jt + 1) * JT],
                                     qt_sb[:, qb * TW:(qb + 1) * TW],
                                     start=True, stop=True)
                    w = wp.tile([128, TW], MMDT, tag="w")
                    nc.scalar.activation(w[:], sp[:], EXP, scale=float(SCALE))
                    if qb < TRI_QB and jt >= 4 * qb:
                        k = jt - 4 * qb
                        nc.gpsimd.affine_select(
                            out=w[:], in_=w[:],
                            compare_op=mybir.AluOpType.is_ge, fill=0.0,
                            base=-JT * k, pattern=[[1, TW]],
                            channel_multiplier=-1)
                    nc.tensor.matmul(avp[:], vaug[:, jt, :], w[:],
                                     start=(i == 0), stop=(i == len(jts) - 1))
                osb = op.tile([H + 1, TW], F32, tag="o")
                nc.vector.tensor_copy(osb[:], avp[:])
                nc.sync.dma_start(out[:, qb * TW:(qb + 1) * TW], osb[:])

            # triangle: x cols [0,2048) feed both KV and Q passes
            for t in range(TRI_QB):
                xtile = load_xtile(t * TW)
                kv_pass(xtile, t)
                q_pass(xtile, t)
                v_trans(t)
                attention(t)
            # rect keys: x cols [4096, 5120)
            for t in range(2):
                xtile = load_xtile(QCOLS + t * TW)
                kv_pass(xtile, TRI_QB + t)
                v_trans(TRI_QB + t)
            # rect queries: x cols [2048, 4096)
            for t in range(TRI_QB, Q_BLOCKS):
                xtile = load_xtile(t * TW)
                q_pass(xtile, t)
                attention(t)

    nc.compile()
    return nc


_NC = None


def _get_nc():
    global _NC
    if _NC is None:
        _NC = _build()
    return _NC


def _in_maps(x, Wk, Wq, Wv):
    wkv = np.concatenate([Wk.T, Wv.T], axis=1).astype(HOST_DT)
    wq = np.ascontiguousarray(Wq.T).astype(HOST_DT)
    eye = np.eye(64, dtype=HOST_DT)
    ident = np.concatenate([eye, eye], axis=0)
    ones = np.ones((128, KVCOLS // JT), HOST_DT)
    maps = []
    for b in range(B):
        xtb = np.ascontiguousarray(x[b].T).astype(HOST_DT)
        xa = np.concatenate([xtb, xtb[:, 0:RK]], axis=1)
        xb = np.concatenate([xtb[:, HALF:], xtb[:, HALF:],
                             xtb[:, RK:HALF]], axis=1)
        for xc in (xa, xb):
            maps.append({"xt": np.ascontiguousarray(xc), "wkv": wkv, "wq": wq,
                         "ident": ident, "ones": ones})
    return maps


def _combine(results, out_dtype):
    out = np.empty((B, T, H), dtype=np.float64)
    for b in range(B):
        ra = results[2 * b]["out"].astype(np.float64)
        rb = results[2 * b + 1]["out"].astype(np.float64)
        # queries [0, 2048): core A triangle is complete
        out[b, :HALF] = (ra[0:H, 0:HALF] / ra[H, 0:HALF]).T
        # queries [2048, 4096): A rect + B triangle + B rect partials
        num = ra[0:H, HALF:] + rb[0:H, 0:HALF] + rb[0:H, HALF:]
        den = ra[H, HALF:] + rb[H, 0:HALF] + rb[H, HALF:]
        out[b, HALF:] = (num / den).T
    return out.astype(out_dtype)


def _run(x, Wk, Wq, Wv, trace=False):
    nc = _get_nc()
    maps = _in_maps(x, Wk, Wq, Wv)
    res = run_bass_kernel_spmd(nc, maps, list(range(NCORES)), trace=trace)
    return _combine(res.results, np.asarray(x).dtype), res


def kernel(x, Wk, Wq, Wv):
    out, _ = _run(x, Wk, Wq, Wv, trace=False)
    return out
